# revision 51
# baseline (speedup 1.0000x reference)
"""CRF (dense projection + Viterbi decode) on 8 Trainium2 NeuronCores.

Strategy: data-parallel over batch (8 batches per core).
Per core:
  Phase 1 (pipelined under the scan): potentials = x @ W in t-chunks of 64
           on the PE; bias (+boundary) applied on the Activation engine
           straight out of PSUM; one scatter DMA per t-chunk lands them in
           the scan layout. The scan starts once t-chunk 0 is resident.
  Phase 2: Viterbi forward scan on the DVE using tensor_reduce with
           apply_transpose (32x32 reshape-block) to reduce over the
           transition-source tag axis that lives on partitions.
           Layout: partition = (j=batch%4, vc=tag&31), free = (q=batch//4,
           vr=tag>>5, ...).
  Phase 3: backpointers in bulk (t-chunks of 8): DVE 32x32 stream
           transpose of the per-step scores, then on the *Pool* engine an
           is_ge compare against the stored per-step maxima, a min()
           against a monotone code row ((127-v)/128 in (0,1], so
           masked-code = min(mask, code)) and one pairwise-max halving;
           DVE finishes with a half-width grouped max-reduce and the
           Activation engine decodes v = 127 - 128*code into uint16.
  Phase 4: bulk 32x32 stream-transpose of the backpointer tile, scatter
           DMAs into per-batch partition rows (brow) and into an
           index-wrapped layout (bpw: u%16 across the 16 partitions of
           each batch group, u>>4 along free) that feeds gpsimd
           indirect_copy composition maps; the suffix map H_t (seeded by
           replicating bp_257 across each batch group and composed via
           the wrapped indices, so all 64 entries stay valid) gives
           path[256] at scan end, letting the 511..257 and 255..0
           backtrace half-chains walk concurrently.

All DMAs ride the SP/Activation hardware DGE; host only shards/reshapes.
"""

import numpy as np
from contextlib import ExitStack

B, T, F, U = 64, 512, 1024, 64
NB = 8  # batches per core
NCORE = 8

_CACHE = {}


def _build_program():
    import concourse.bass as bass
    import concourse.bacc as bacc
    import concourse.mybir as mybir
    import concourse.tile as tile

    dt = mybir.dt
    AL = mybir.AluOpType
    AX = mybir.AxisListType
    ACT = mybir.ActivationFunctionType

    nc = bacc.Bacc("TRN2", target_bir_lowering=False, debug=False,
                   num_devices=NCORE)

    # ---- DRAM I/O ----
    d_xt = nc.dram_tensor("xt", [NB, F, T], dt.float32, kind="ExternalInput")
    d_wkp = nc.dram_tensor("wkp", [128, 512], dt.float32, kind="ExternalInput")
    d_bias = nc.dram_tensor("biasc", [64, 1], dt.float32, kind="ExternalInput")
    d_lb = nc.dram_tensor("lbc", [64, 1], dt.float32, kind="ExternalInput")
    d_rb = nc.dram_tensor("rbc", [64, 1], dt.float32, kind="ExternalInput")
    d_crep = nc.dram_tensor("crep", [128, 128], dt.float32, kind="ExternalInput")
    d_iot = nc.dram_tensor("iot", [128, 64], dt.float32, kind="ExternalInput")
    d_tags = nc.dram_tensor("tags", [NB, T], dt.int32, kind="ExternalOutput")

    TB = 513          # beta t-slots (1..512 used)

    with tile.TileContext(nc) as tc, ExitStack() as ctx:
        cpool = ctx.enter_context(tc.tile_pool(name="consts", bufs=1))
        st = ctx.enter_context(tc.tile_pool(name="state", bufs=1))
        xpool = ctx.enter_context(tc.tile_pool(name="xs", bufs=4))
        bpool = ctx.enter_context(tc.tile_pool(name="pb", bufs=2))
        ppool = ctx.enter_context(tc.tile_pool(name="ps", bufs=8, space="PSUM"))

        wk = cpool.tile([128, 512], dt.float32, tag="wk")
        crep = cpool.tile([128, 128], dt.float32, tag="crep")
        iot = cpool.tile([128, 64], dt.float32, tag="iot")
        iotH = cpool.tile([128, 64], dt.bfloat16, tag="iotH")
        biasc = cpool.tile([64, 1], dt.float32, tag="biasc")
        lbc = cpool.tile([64, 1], dt.float32, tag="lbc")
        rbc = cpool.tile([64, 1], dt.float32, tag="rbc")
        biasL = cpool.tile([64, 1], dt.float32, tag="biasL")
        biasR = cpool.tile([64, 1], dt.float32, tag="biasR")
        zcol = cpool.tile([128, 1], dt.float32, tag="zcol")
        zidx = cpool.tile([128, 1], dt.uint16, tag="zidx")

        pot = st.tile([128, 2048], dt.float32, tag="pot")    # (q,uh,t<512)
        # gam2[(j,x), 2048q + 1024uh + 512vr + t] = gamma_t[b=(q,j),
        #   tag=(vr,x)] replicated over uh so the scan add reads (uh,vr) as
        #   one stride-512 dim.
        gam2 = st.tile([128, 4096], dt.float32, tag="gam2")
        beta = st.tile([128, 2 * 2 * TB], dt.float32, tag="beta")  # (q,uh,t)
        # bpu[(j,uc), 1024q + 64*((t-1)>>5) + 32uh + ((t-1)&31)]
        bpu = st.tile([128, 2048], dt.uint16, tag="bpu")
        # ssring slot cols: 128q + 64uh + 32vr + uc (per-step scores);
        # 24 slots = 3 chunks so the transpose can lag 2 chunks.
        ssring = st.tile([128, 24 * 256], dt.float32, tag="ssring")
        stts = [st.tile([128, 2048], dt.float32, tag=f"stt{i}", name=f"stt{i}")
                for i in range(2)]
        pkks = [st.tile([128, 2048], dt.bfloat16, tag=f"pkk{i}", name=f"pkk{i}")
                for i in range(2)]
        pk2s = [st.tile([128, 2048], dt.bfloat16, tag=f"pk2{i}", name=f"pk2{i}")
                for i in range(3)]
        pk3s = [st.tile([128, 1024], dt.bfloat16, tag=f"pk3{i}", name=f"pk3{i}")
                for i in range(2)]
        pk4s = [st.tile([128, 512], dt.bfloat16, tag=f"pk4{i}", name=f"pk4{i}")
                for i in range(2)]
        pk5s = [st.tile([128, 256], dt.bfloat16, tag=f"pk5{i}", name=f"pk5{i}")
                for i in range(2)]
        r4 = st.tile([128, 64], dt.float32, tag="r4")
        btp_all = st.tile([128, 2048], dt.uint16, tag="btp_all")
        # brow[16b, 1024*((t-1)&31) + 64*((t-1)>>5) + u] = bp[b, t, u]
        brow = st.tile([128, 32 * 1024], dt.uint16, tag="brow")
        # hs: ping-pong suffix maps H_t = bp_257 o ... o bp_t (uint16 tags)
        hs = [st.tile([128, 64], dt.uint16, tag=f"hs{i}", name=f"hs{i}")
              for i in range(2)]
        tags16 = st.tile([128, 4 * 512], dt.uint16, tag="tags16")
        tagsi = st.tile([128, 512], dt.int32, tag="tagsi")

        def A(tl, p0, np_, f0, dims):
            full = tl[:]
            pitch = full.ap[0][0]
            return bass.AP(full.tensor, full.offset + p0 * pitch + f0,
                           [[pitch, np_]] + [list(d) for d in dims])

        # ---- constant loads / inits ----
        nc.sync.dma_start(wk[:], d_wkp[:])
        nc.sync.dma_start(crep[:], d_crep[:])
        nc.sync.dma_start(iot[:], d_iot[:])
        nc.sync.dma_start(biasc[:], d_bias[:])
        nc.sync.dma_start(lbc[:], d_lb[:])
        nc.sync.dma_start(rbc[:], d_rb[:])
        nc.vector.memset(zcol[:], 0.0)
        nc.vector.tensor_copy(out=iotH[:], in_=iot[:])
        nc.vector.memset(zidx[:], 0)
        nc.vector.memset(tags16[:], 0)
        nc.vector.tensor_add(biasL[:], biasc[:], lbc[:])
        nc.vector.tensor_add(biasR[:], biasc[:], rbc[:])

        # ---- phase 1 (t-chunked, pipelined under the scan) ----
        warm = {"done": False}
        pb = {}

        def p1_load(tcn, b):
            xt = xpool.tile([128, 512], dt.float32, tag="xt")
            src = bass.AP(d_xt[:].tensor,
                          d_xt[:].offset + b * (F * T) + 64 * tcn,
                          [[T, 128], [128 * T, 8], [1, 64]])
            nc.sync.dma_start(xt[:], src)
            return xt

        def p1_mm(tcn, b, xt):
            if b == 0:
                potb_new = bpool.tile([64, 512], dt.float32, tag="potb",
                                      name=f"potb{tcn}")
                pb["cur"] = potb_new
            potb = pb["cur"]
            pp = ppool.tile([64, 64], dt.float32, tag="pp")
            if not warm["done"]:
                # PE warmup; folds the wk-DMA dependency into PE program
                # order so later matmuls need only their x-tile DMA wait.
                nc.tensor.matmul(pp[0:64, 0:1], wk[:, 0:64], wk[:, 0:1],
                                 start=True, stop=True)
                warm["done"] = True
            for kc in range(8):
                nc.tensor.matmul(pp[:], wk[:, kc * 64:(kc + 1) * 64],
                                 xt[:, kc * 64:(kc + 1) * 64],
                                 start=(kc == 0), stop=(kc == 7))
            if tcn == 0:
                nc.scalar.activation(potb[:, 64 * b:64 * b + 1], pp[:, 0:1],
                                     ACT.Identity, bias=biasL[:])
                nc.scalar.activation(potb[:, 64 * b + 1:64 * b + 64],
                                     pp[:, 1:64], ACT.Identity, bias=biasc[:])
            elif tcn == 7:
                nc.scalar.activation(potb[:, 64 * b:64 * b + 63], pp[:, 0:63],
                                     ACT.Identity, bias=biasc[:])
                nc.scalar.activation(potb[:, 64 * b + 63:64 * b + 64],
                                     pp[:, 63:64], ACT.Identity, bias=biasR[:])
            else:
                nc.scalar.activation(potb[:, 64 * b:64 * b + 64], pp[:],
                                     ACT.Identity, bias=biasc[:])

        def p1_scatter(tcn):
            # pot[(j,uc), 1024q + 512uh + 64tc + i] = potb[32uh+uc, 64b+i].
            # Partition dims stay dim-0 on both sides (partition-crossing
            # free dims are invisible to the dependency tracker).
            pbv = pb["cur"][:]
            pp_ = pbv.ap[0][0]
            for uh in range(2):
                for j in range(4):
                    src = bass.AP(pbv.tensor,
                                  pbv.offset + 32 * uh * pp_ + 64 * j,
                                  [[pp_, 32], [256, 2], [1, 64]])
                    dst = A(pot, 32 * j, 32, 512 * uh + 64 * tcn,
                            [[1024, 2], [1, 64]])
                    nc.sync.dma_start(dst, src)

        # head: t-chunk 0 resident before the scan starts; tc1 b0/b1 early
        xts = [p1_load(0, b) for b in range(NB)]
        for b in range(NB):
            p1_mm(0, b, xts[b])
        p1_scatter(0)
        xts[0] = p1_load(1, 0)
        xts[1] = p1_load(1, 1)
        p1_mm(1, 0, xts[0])
        p1_mm(1, 1, xts[1])

        # brow junk-row guard (rows 16b+1..15 are read by the link gathers'
        # index unwrap): zero it on the Activation engine, split so the
        # queue drains before the first decode matters.
        br = brow[:]
        zb = bass.AP(zcol[:].tensor, zcol[:].offset,
                     [[zcol[:].ap[0][0], 128], [0, 4096]])
        for i in range(8):
            nc.scalar.activation(
                bass.AP(br.tensor, br.offset + 4096 * i,
                        [[br.ap[0][0], 128], [1, 4096]]),
                zb, ACT.Copy)

        # gamma_0 = pot_0 (replicated over uh)
        nc.vector.tensor_copy(
            out=A(gam2, 0, 128, 0, [[2048, 2], [512, 2], [1024, 2]]),
            in_=A(pot, 0, 128, 0, [[1024, 2], [512, 2], [0, 2]]))

        # ---- phase 2/3 chunk machinery ----
        def bp_transpose(ci, t0, L):
            stt = stts[ci % 2]
            rbase = ((t0 - 1) % 24) * 256
            for th in range(4):
                for qq in range(2):
                    off = 512 * th + qq * 128
                    nc.vector.transpose(
                        out=A(stt, 0, 128, off, [[256, L // 4], [1, 128]]),
                        in_=A(ssring, 0, 128, rbase + off,
                              [[256, L // 4], [1, 128]]))

        def bp_cmp(ci, t0, L, qq):
            # is_ge mask on the DVE (Pool's TT ALU only has mult/add/sub),
            # in t-halves for finer gap-filling
            stt = stts[ci % 2]
            pkk = pkks[ci % 2]
            for th in range(4):
                off = 512 * th + qq * 128
                s_in = A(stt, 0, 128, off, [[256, L // 4], [64, 2], [1, 64]])
                b_in = A(beta, 0, 128, 2 * TB * qq + t0 + 2 * th,
                         [[1, L // 4], [TB, 2], [0, 64]])
                p_out = A(pkk, 0, 128, off, [[256, L // 4], [64, 2], [1, 64]])
                nc.vector.tensor_tensor(out=p_out, in0=s_in, in1=b_in,
                                        op=AL.is_ge)

        def bp_min(ci, t0, L, qq):
            # masked code on the DVE at the 2x bf16 rate: codes iotH =
            # (127-v)/128 lie in (0,1] and the mask is {0,1}, so
            # min(mask, code) == mask*code; all-DVE keeps the chunk
            # pipeline free of cross-engine roundtrips.
            pkk = pkks[ci % 2]
            pk2 = pk2s[ci % 3]
            for th in range(4):
                off = 512 * th + qq * 128
                nc.vector.tensor_tensor(
                    out=A(pk2, 0, 128, off, [[256, L // 4], [64, 2], [1, 64]]),
                    in0=A(pkk, 0, 128, off, [[256, L // 4], [64, 2], [1, 64]]),
                    in1=A(iotH, 0, 128, 0, [[0, L // 4], [0, 2], [1, 64]]),
                    op=AL.min)

        def bp_halve(ci, t0, L, lvl, qq=None):
            # bf16 pairwise-max halvings at 2x DVE rate; any reduction
            # tree picks the same winner (codes globally distinct).
            src = [pk2s, pk3s, pk4s][lvl][ci % (3 if lvl == 0 else 2)]
            dst = [pk3s, pk4s, pk5s][lvl][ci % 2]
            w = 32 >> lvl                   # surviving half-width
            if qq is None:
                nc.vector.tensor_tensor(
                    out=A(dst, 0, 128, 0, [[w, 4 * L], [1, w]]),
                    in0=A(src, 0, 128, 0, [[2 * w, 4 * L], [1, w]]),
                    in1=A(src, 0, 128, w, [[2 * w, 4 * L], [1, w]]),
                    op=AL.max)
            else:
                # qq-half: groups (t, uh) at col 4wt + 2w*qq + w*uh in src
                nc.vector.tensor_tensor(
                    out=A(dst, 0, 128, 2 * w * qq,
                          [[4 * w, L], [w, 2], [1, w]]),
                    in0=A(src, 0, 128, 4 * w * qq,
                          [[8 * w, L], [2 * w, 2], [1, w]]),
                    in1=A(src, 0, 128, 4 * w * qq + w,
                          [[8 * w, L], [2 * w, 2], [1, w]]),
                    op=AL.max)

        def bp_dec(ci, t0, L, qq):
            pk5 = pk5s[ci % 2]
            # r4 cols 4t + 2q + uh
            nc.vector.tensor_reduce(
                A(r4, 0, 128, 2 * qq, [[4, L], [1, 2]]),
                A(pk5, 0, 128, 16 * qq, [[32, L], [8, 2], [1, 8]]),
                AX.X, AL.max)

        def bp_decode(ci, t0, L):
            tc0, rr0 = (t0 - 1) >> 5, (t0 - 1) & 31
            nc.scalar.activation(
                A(bpu, 0, 128, 64 * tc0 + rr0, [[1, L], [1024, 2], [32, 2]]),
                A(r4, 0, 128, 0, [[1, 4 * L]]),
                ACT.Copy, bias=127.0, scale=-128.0)

        brp = brow[:].ap[0][0]
        btpp = btp_all[:].ap[0][0]

        def block_slice(t):
            off = (brow[:].offset + ((t - 1) & 31) * 1024
                   + ((t - 1) >> 5) * 64)
            return bass.AP(brow[:].tensor, off, [[brp, 128], [1, 64]])

        def emit_block(k):
            """Transpose bpu's tc-block k into btp_all, scatter it into the
            per-batch brow rows and the wrapped bpw layout."""
            dims = [[1024, 2], [32, 2], [1, 32]]
            nc.vector.transpose(out=A(btp_all, 0, 128, 64 * k, dims),
                                in_=A(bpu, 0, 128, 64 * k, dims))
            for b in range(NB):
                q, j = b >> 2, b & 3
                src = bass.AP(btp_all[:].tensor,
                              btp_all[:].offset + 32 * j * btpp
                              + 1024 * q + 64 * k,
                              [[btpp, 32], [1, 64]])
                dst = bass.AP(brow[:].tensor,
                              brow[:].offset + 16 * b * brp + 64 * k,
                              [[brp, 1], [1024, 32], [1, 64]])
                nc.sync.dma_start(dst, src)

        def h_gather(t):
            # H_t = H_{t-1} o bp_t (H_258 seeds from bp_257 directly)
            data = block_slice(257) if t == 258 else hs[(t - 1) & 1][:]
            nc.gpsimd.indirect_copy(hs[t & 1][:], data, block_slice(t), True)

        # ---- phase 2: forward scan with interleaved chunk machinery ----
        hstate = {"t": 258}

        def h_links(ci, n):
            k_done = (ci - 7) // 4          # brow blocks scattered so far
            t_lim = 32 * k_done + 32
            for _ in range(n):
                if hstate["t"] > 480 or hstate["t"] > t_lim:
                    break
                h_gather(hstate["t"])
                hstate["t"] += 1

        for t in range(1, 513):
            ci = (t - 1) >> 3
            p = (t - 1) & 7
            slot = (t - 1) % 24
            sbase = slot * 256
            # scores = gamma + C, split by q-half: q0 on the DVE, q1 on
            # the Pool engine so the two halves overlap.
            for q in range(2):
                if t <= 511:
                    in0 = A(crep, 0, 128, 0, [[1, 128]])
                else:
                    in0 = A(zcol, 0, 128, 0, [[0, 128]])
                in1 = A(gam2, 0, 128, 2048 * q + t - 1, [[512, 4], [0, 32]])
                eng = nc.vector if q == 0 else nc.gpsimd
                eng.tensor_tensor(
                    out=ssring[:, sbase + 128 * q:sbase + 128 * q + 128],
                    in0=in0, in1=in1, op=AL.add)
            rb = 32 + 8 * (t & 1)           # rr ping-pong in r4 cols 32..47
            nc.vector.tensor_reduce(A(r4, 0, 128, rb, [[1, 8]]),
                                    A(ssring, 0, 128, sbase, [[32, 8], [1, 32]]),
                                    AX.X, AL.max, apply_transpose=True)
            # rr cols: (q, uh, vr)
            bsl = A(beta, 0, 128, t, [[2 * TB, 2], [TB, 2]])
            nc.vector.tensor_tensor(out=bsl,
                                    in0=A(r4, 0, 128, rb, [[4, 2], [2, 2]]),
                                    in1=A(r4, 0, 128, rb + 1, [[4, 2], [2, 2]]),
                                    op=AL.max)
            if t <= 511:
                # gam2 = beta + pot on the (body-idle) Pool engine; the
                # next step's scores TT waits on its semaphore.
                nc.gpsimd.tensor_tensor(
                    out=A(gam2, 0, 128, t,
                          [[2048, 2], [512, 2], [1024, 2]]),
                    in0=A(beta, 0, 128, t, [[2 * TB, 2], [TB, 2], [0, 2]]),
                    in1=A(pot, 0, 128, t, [[1024, 2], [512, 2], [0, 2]]),
                    op=AL.add)

            # interleaved chunk machinery / phase-1 pipeline
            s = ci + 10                     # phase-1 lead of 10 chunks
            if p == 0 and ci >= 3:
                bp_transpose(ci - 2, 8 * ci - 15, 8)
            elif p == 1 and ci >= 3:
                bp_cmp(ci - 2, 8 * ci - 15, 8, 0)
                bp_min(ci - 2, 8 * ci - 15, 8, 0)
            elif p == 2 and ci >= 3:
                bp_cmp(ci - 2, 8 * ci - 15, 8, 1)
                bp_min(ci - 2, 8 * ci - 15, 8, 1)
            elif p == 4 and ci == 1:
                # chunk 0's machinery one chunk early: densifies the ramp
                bp_transpose(0, 1, 8)
                bp_cmp(0, 1, 8, 0)
                bp_min(0, 1, 8, 0)
                bp_cmp(0, 1, 8, 1)
                bp_min(0, 1, 8, 1)
            elif p == 3 and ci >= 3:
                bp_halve(ci - 3, 8 * ci - 23, 8, 0, 0)
                bp_halve(ci - 3, 8 * ci - 23, 8, 0, 1)
                bp_halve(ci - 3, 8 * ci - 23, 8, 1)
                if (ci - 7) % 4 == 0 and ci >= 7:
                    emit_block((ci - 7) // 4)
            elif p == 4 and ci >= 3:
                bp_halve(ci - 3, 8 * ci - 23, 8, 2)
                bp_dec(ci - 3, 8 * ci - 23, 8, 0)
                bp_dec(ci - 3, 8 * ci - 23, 8, 1)
                bp_decode(ci - 3, 8 * ci - 23, 8)
            elif p == 5 and 10 <= s <= 63:
                xts[s % 8] = p1_load(s // 8, s % 8)
            elif p == 6 and 10 <= s <= 63:
                p1_mm(s // 8, s % 8, xts[s % 8])
            elif p == 7 and 10 <= s <= 63 and s % 8 == 7:
                p1_scatter(s // 8)
            if p == 7 and ci >= 40:
                h_links(ci, 10)

        # ---- post-loop: finish chunks 60..63 (incl. the t=512 probe) ----
        def bp_finish(c):
            t0 = 8 * c + 1
            bp_halve(c, t0, 8, 0, 0)
            bp_halve(c, t0, 8, 0, 1)
            bp_halve(c, t0, 8, 1)
            bp_halve(c, t0, 8, 2)
            bp_dec(c, t0, 8, 0)
            bp_dec(c, t0, 8, 1)
            bp_decode(c, t0, 8)

        # finish 60/61 first (60 must drain pk2s[0] before chunk 63 == 60
        # mod 3 rewrites it), then emit block 14 early so its H-map links
        # (t=449..480) run on Pool underneath the remaining DVE finish work.
        bp_finish(61)
        while hstate["t"] <= 480:
            h_gather(hstate["t"])
            hstate["t"] += 1
        bp_transpose(62, 497, 8)
        for qq in range(2):
            bp_cmp(62, 497, 8, qq)
            bp_min(62, 497, 8, qq)
        bp_transpose(63, 505, 8)
        for qq in range(2):
            bp_cmp(63, 505, 8, qq)
            bp_min(63, 505, 8, qq)
        bp_finish(62)
        bp_finish(63)
        emit_block(15)

        # ---- phase 4: two interleaved backtrace half-chains (gpsimd) ----
        def link(t):
            # path[t-1] = bp_t[path[t]]
            nc.gpsimd.indirect_copy(tags16[:, 4 * (t - 1):4 * t],
                                    block_slice(t),
                                    tags16[:, 4 * t:4 * t + 1], True)

        nc.gpsimd.indirect_copy(tags16[:, 4 * 511:4 * 512], block_slice(512),
                                zidx[:], True)
        up_t = 511
        # finish the H chain interleaved with upper links
        for t in range(hstate["t"], 513):
            h_gather(t)
            link(up_t)
            up_t -= 1
        # path[256] = H_512[*] (every entry equals H_511[last_tag])
        nc.gpsimd.indirect_copy(tags16[:, 4 * 256:4 * 256 + 4],
                                hs[512 & 1][:], zidx[:], True)
        lo_t = 256
        while up_t >= 258 or lo_t >= 1:
            if up_t >= 258:
                link(up_t)
                up_t -= 1
            if lo_t >= 1:
                link(lo_t)
                lo_t -= 1

        t16 = tags16[:]
        nc.vector.tensor_copy(
            out=tagsi[:],
            in_=bass.AP(t16.tensor, t16.offset, [[t16.ap[0][0], 128], [4, 512]]))
        ti = tagsi[:]
        src = bass.AP(ti.tensor, ti.offset, [[16 * ti.ap[0][0], 8], [1, 512]])
        nc.sync.dma_start(d_tags[:], src)

    nc.finalize()
    return nc


def _host_inputs(x, kernel, bias, chain_kernel, left_boundary, right_boundary):
    """Build per-core input maps (host does only sharding/layout)."""
    x = np.asarray(x, dtype=np.float32)
    wk = np.asarray(kernel, dtype=np.float32)
    bias = np.asarray(bias, dtype=np.float32)
    C = np.asarray(chain_kernel, dtype=np.float32)
    lb = np.asarray(left_boundary, dtype=np.float32)
    rb = np.asarray(right_boundary, dtype=np.float32)

    wkp = wk.reshape(8, 128, 64).transpose(1, 0, 2).reshape(128, 512).copy()
    # crep[32j+vc, 64uh+32vr+uc] = C[32vr+vc, 32uh+uc]
    crep = np.tile(
        C.reshape(2, 32, 2, 32).transpose(1, 2, 0, 3).reshape(32, 128),
        (4, 1)).copy()
    iota_row = (127.0 - np.arange(64, dtype=np.float32)) / 128.0
    iot = np.tile(iota_row[None, :], (128, 1)).copy()
    biasc = bias.reshape(64, 1).copy()
    lbc = lb.reshape(64, 1).copy()
    rbc = rb.reshape(64, 1).copy()

    in_maps = []
    for c in range(NCORE):
        xc = x[c * NB:(c + 1) * NB]                       # [8, 512, 1024]
        xt = np.ascontiguousarray(xc.transpose(0, 2, 1))  # [8, 1024, 512]
        in_maps.append({
            "xt": xt, "wkp": wkp, "biasc": biasc, "lbc": lbc, "rbc": rbc,
            "crep": crep, "iot": iot,
        })
    return in_maps


def kernel(x, kernel, bias, chain_kernel, left_boundary, right_boundary):
    from concourse.bass_utils import run_bass_kernel_spmd

    if "nc" not in _CACHE:
        _CACHE["nc"] = _build_program()
    nc = _CACHE["nc"]

    in_maps = _host_inputs(x, kernel, bias, chain_kernel,
                           left_boundary, right_boundary)
    res = run_bass_kernel_spmd(nc, in_maps, core_ids=list(range(NCORE)))
    outs = [np.asarray(r["tags"]).astype(np.int32) for r in res.results]
    return np.concatenate(outs, axis=0)
